# revision 37
# baseline (speedup 1.0000x reference)
"""Trainium2 Bass kernel for the DMP-rollout Net (nn_Net_60567628808344).

Math
----
The reference integrates, per row r of p = (x*scale).reshape(-1, 27):
    y0 = p[:,0], goal = p[:,1], w = p[:,2:]
    cx_j = (1 - A_X*DT/TAU)^j                     (data independent)
    psi_j = exp(-0.5 (cx_j - c)^2 / sigma2)       (data independent)
    state update is LINEAR:  s_j = M s_{j-1} + [0; k*(Az*Bz*goal + F_j)]
    with F_j = (w @ psi_j) * cx_j * (goal-y0) / sum(psi_j)
So the whole 301-step rollout collapses to a closed form
    out[r, i] = A[i]*y0_r + B[i]*goal_r + (goal_r - y0_r) * (w_r @ H[i, :])
with constant A,B (301,), H (301, 25) precomputed in float64 on host.

Device work per core (8-way batch data-parallel) is then a single
(16384 x 27) @ (27 x 301) matmul + the 19.7 MB/core output writeback,
which is HBM-write roofline bound (~430 GB/s across the 16 DMA engines).

Inputs are bf16 (tolerance is 2e-2; bf16 keeps rel err ~5e-3) so the PE
produces well above the DMA drain rate, and the input load halves.

The device writes the output in bf16 (9.9 MB/core instead of 19.7; the
host casts back to fp32 after the gather -- adds ~0.4% rounding, total
rel err ~9e-3 vs the 2e-2 gate), which halves the HBM-write roofline.

PSUM is one persistent [128, 8, 512] tile (all 8 banks); matmul for tile
k writes bank k%8. PSUM->SBUF copies (with fp32->bf16 cast) move TWO
tiles per instruction via one strided read across 2 banks (4 units live
across 8 banks kills the copy->matmul->copy bank round-trip). A host-
side row permutation decouples production order from HBM order: each
output DMA group is a contiguous single-ENGINE run in stage space (one
sync wait per dma, the walrus ISA limit), while in time the DVE and ACT
chains interleave (GPSIMD cannot read PSUM). Production is the floor:
both copy engines run gap-free at instruction throughput.

Layout: per-core rows are packed 4 tiles at a time into the partition
dim (4 groups of 32 partitions, K padded 27->32 with zeros) so each
quad issues 4 row-group-packed concurrent matmuls (tile_position
(32q, 0) auto-derived from base partitions).
"""

import numpy as np
import ml_dtypes

# DMP hyperparameters fixed by Net.__init__ (hardcoded per problem spec)
N = 25
DOF = 2
DT = 0.01
TAU = 3.0
A_X = 2.0
A_Z = 48.0
B_Z = A_Z / 4.0
T = 301                    # time steps
BATCH = 65536
PARAM_DIM = DOF * (N + 2)  # 54
NCORES = 8

ROWS = BATCH * DOF         # 131072 (B*DOF rows)
RPC = ROWS // NCORES       # 16384 rows per core
TILES = RPC // 128         # 128 tiles of 128 rows per core
UNITS = TILES // 2         # 64 copy units of 2 tiles (half a j iteration)
QUAD_COLS = RPC // 4       # 4096: vt free dim (4 tiles packed in partition dim)

# Copy units are 2 tiles (2 PSUM banks): 4 units live across the 8 banks
# breaks the copy->matmul->copy bank round-trip that 4-tile units suffer
# (only 2 in flight), so each engine's copies run back-to-back.
#
# Production order is DECOUPLED from HBM order by a host-side row
# permutation: each output DMA group is a contiguous single-ENGINE run in
# stage space (single sync wait, no dma pairing), while in time the copy
# engines (DVE ~0.79us, ACT ~0.71us per 2-tile unit) work interleaved.
# GPSIMD cannot access PSUM (BIR verifier), so only DVE/ACT copy. 7 finer
# single-writer groups beat 4 paired ones: the last group is small, so
# the tail drain after production-end shrinks. Sizes from the gate model.
GROUPS = ((1, 2), (0, 10), (1, 14), (0, 10), (1, 9), (0, 9), (1, 10))
assert sum(n for _, n in GROUPS) == UNITS

# per-engine totals and stage-slot lists
_slots = {0: [], 1: [], 2: []}
_s0 = 0
for _e, _n in GROUPS:
    _slots[_e] += list(range(_s0, _s0 + _n))
    _s0 += _n
_TOT = {e: len(s) for e, s in _slots.items()}
# production order: weighted round-robin (Bresenham) over engines
_sched = sorted(
    ((k + 0.5) * UNITS / _TOT[e], e) for e in _slots for k in range(_TOT[e])
)
PROD_ENG = tuple(e for _, e in _sched)          # engine of production unit u
_ord = {0: 0, 1: 0, 2: 0}
PROD_STAGE = []                                  # stage slot of prod unit u
for _e in PROD_ENG:
    PROD_STAGE.append(_slots[_e][_ord[_e]])
    _ord[_e] += 1
PROD_STAGE = tuple(PROD_STAGE)
# dma trigger: group g fires after the production unit that completes it
_cum = {0: 0, 1: 0, 2: 0}
_fire = {}
_s0 = 0
for _gi, (_e, _n) in enumerate(GROUPS):
    _cum[_e] += _n
    _last_prod = [u for u in range(UNITS)
                  if PROD_ENG[u] == _e][_cum[_e] - 1]
    _fire.setdefault(_last_prod, []).append((_s0, _s0 + _n))
    _s0 += _n
DMA_FIRE = {u: tuple(v) for u, v in _fire.items()}
# tile-level permutation for host packing: production tile k -> stage tile
ST_TILE = tuple(2 * PROD_STAGE[k // 2] + k % 2 for k in range(TILES))

A_QUADS = 2                # quads in the tiny head chunk (with hc)
B_QUADS = 4                # quads in the second chunk (SWDGE)


# ----------------------------------------------------------------------------
# Host-side constant build (exact, float64)
# ----------------------------------------------------------------------------
_const_cache = {}


def _build_constants(c=None, sigma2=None):
    """Return hc (128, 301) bf16: rows 32q+p hold
    p==0 -> A, p==1 -> B, p==2+n -> H[:, n], rows 27..31 of each group zero."""
    if c is None:
        c = np.exp(-A_X * np.linspace(0.0, 1.0, N))
    if sigma2 is None:
        sigma2 = (N ** 1.5) / c / A_X
    c = np.asarray(c, np.float64)
    sigma2 = np.asarray(sigma2, np.float64)
    key = (c.tobytes(), sigma2.tobytes())
    if key in _const_cache:
        return _const_cache[key]

    k = DT / TAU
    M = np.array([[1.0, k], [-A_Z * B_Z * k, 1.0 - A_Z * k]])
    P = np.zeros(T + 1)
    Q = np.zeros(T + 1)
    Mn = np.eye(2)
    for n in range(T + 1):
        P[n] = Mn[0, 0]
        Q[n] = Mn[0, 1]
        Mn = Mn @ M

    decay = 1.0 - A_X * DT / TAU
    cx = decay ** np.arange(1, T + 1)                        # cx_1..cx_T
    psi = np.exp(-0.5 * (cx[:, None] - c[None, :]) ** 2 / sigma2[None, :])
    g = psi * (cx / psi.sum(1))[:, None]                     # (T, N)

    A = P[1:T + 1]
    B = k * A_Z * B_Z * np.cumsum(Q[0:T])
    # H[i] = k * sum_{m<=i} Q[i-m] g[m]  -- lower-triangular Toeplitz matvec
    ii = np.arange(T)[:, None]
    mm = np.arange(T)[None, :]
    L = np.where(ii >= mm, Q[np.clip(ii - mm, 0, T)], 0.0)   # (T, T)
    H = k * (L @ g)                                          # (T, N)

    hfull = np.zeros((32, T), np.float32)
    hfull[0] = A.astype(np.float32)
    hfull[1] = B.astype(np.float32)
    hfull[2:2 + N] = H.T.astype(np.float32)
    hc = np.tile(hfull, (4, 1)).astype(ml_dtypes.bfloat16)   # (128, T)
    _const_cache[key] = hc
    return hc


def _pack_inputs(x, c, sigma2, scale):
    """Build per-core vt arrays (128, 4096) bf16 + shared hc (128, 301)."""
    x = np.asarray(x, np.float32)
    if scale is None:
        scale = np.ones(PARAM_DIM, np.float32)
    p = (x * np.asarray(scale, np.float32)).reshape(ROWS, N + 2)
    y0 = p[:, 0]
    goal = p[:, 1]
    u = goal - y0
    v = np.empty((ROWS, N + 2), np.float32)
    v[:, 0] = y0
    v[:, 1] = goal
    v[:, 2:] = p[:, 2:] * u[:, None]
    v = v.astype(ml_dtypes.bfloat16)

    hc = _build_constants(c, sigma2)

    vts = []
    for i in range(NCORES):
        vc = v[RPC * i:RPC * (i + 1)]                 # (16384, 27)
        # Tile t=4j+q, lhsT free index f computes local row 128*f + t, so
        # each out-partition owns a contiguous run of HBM rows (linear
        # writeback).  row = 128*f + 4*j + q -> [f, j, q, p] -> [q, p, j, f]
        vperm = vc.reshape(128, TILES, N + 2)[:, ST_TILE, :]
        v4 = vperm.reshape(128, TILES // 4, 4, N + 2).transpose(2, 3, 1, 0)
        vp = np.zeros((4, 32, TILES // 4, 128), ml_dtypes.bfloat16)
        vp[:, :N + 2] = v4
        vts.append(np.ascontiguousarray(vp.reshape(128, QUAD_COLS)))
    return vts, hc


# ----------------------------------------------------------------------------
# Bass kernel
# ----------------------------------------------------------------------------
_nc_cache = []


def _build_bass():
    if _nc_cache:
        return _nc_cache[0]
    import concourse.bass as bass
    import concourse.mybir as mybir
    from concourse import tile
    import bass_rust
    from concourse.vector_clock import ScopedClock

    class SplitDrainTileContext(tile.TileContext):
        """This walrus build allows a single sync wait per instruction, but
        TileContext's kernel-tail drain carries one wait per live sem lane.
        Split the extras onto standalone single-wait SP nops (same stream, so
        all waits still complete before the barrier + sem clearing)."""

        def _drain_and_barrier(self, tick_clock, wait_clock):
            nc = self.nc
            drain_inst = nc.sync.drain()
            wait_clock.add_sem_waits(
                drain_inst.ins, ScopedClock({None: tick_clock.global_clock})
            )
            si = drain_inst.ins.sync_info
            waits = list(si.on_wait) if si is not None else []
            if len(waits) > 1:
                drain_inst.ins.sync_info = bass_rust.SyncInfo(
                    on_wait=[waits[0]], on_update=list(si.on_update)
                )
                engs = [nc.vector, nc.scalar, nc.tensor, nc.gpsimd, nc.sync]
                for wi, w in enumerate(waits[1:]):
                    n = engs[wi % len(engs)].nop(nofuse=True)
                    n.ins.sync_info = bass_rust.SyncInfo(
                        on_wait=[w], on_update=[]
                    )
            nc.all_engine_barrier()
            assert self.sems is not None
            popped = nc._tile_sem_poison_stack.pop()
            assert popped is self._sem_poison
            nc.clear_and_free_semaphores(list(self.sems.allocated().values()))
            nc.all_engine_barrier()

    f32 = mybir.dt.float32
    fmm = mybir.dt.bfloat16
    nc = bass.Bass()
    # Input split: a small head chunk (first A_QUADS quads + the 301 constant
    # columns) so compute starts early, the rest in parallel. Both ride the
    # gpsimd SWDGE so all 8 HWDGE sem lanes are free for output groups.
    va_d = nc.dram_tensor("va", [128, 128 * A_QUADS + T], fmm, kind="ExternalInput")
    vm_d = nc.dram_tensor("vm", [128, 128 * B_QUADS], fmm, kind="ExternalInput")
    vb_d = nc.dram_tensor(
        "vb", [128, QUAD_COLS - 128 * (A_QUADS + B_QUADS)], fmm,
        kind="ExternalInput"
    )
    out_d = nc.dram_tensor("out", [RPC, T], fmm, kind="ExternalOutput")

    with SplitDrainTileContext(nc) as tc:
        with (
            tc.tile_pool(name="vtp", bufs=1) as vtp,
            tc.tile_pool(name="stage", bufs=1) as stagep,
            tc.tile_pool(name="psum", bufs=1, space="PSUM") as psump,
        ):
            vtsA = vtp.tile([128, 128 * A_QUADS + T], fmm, tag="vtsA")
            vtsM = vtp.tile([128, 128 * B_QUADS], fmm, tag="vtsM")
            vtsB = vtp.tile(
                [128, QUAD_COLS - 128 * (A_QUADS + B_QUADS)], fmm, tag="vtsB"
            )
            nc.scalar.dma_start(vtsA[:], va_d[:])
            nc.gpsimd.dma_start(vtsM[:], vm_d[:])
            nc.gpsimd.dma_start(vtsB[:], vb_d[:])
            hrep = vtsA[:, 128 * A_QUADS:128 * A_QUADS + T]

            def lhsT(j, q):
                if j < A_QUADS:
                    return vtsA[32 * q:32 * q + 32, 128 * j:128 * (j + 1)]
                if j < A_QUADS + B_QUADS:
                    jm = j - A_QUADS
                    return vtsM[32 * q:32 * q + 32, 128 * jm:128 * (jm + 1)]
                jb = j - A_QUADS - B_QUADS
                return vtsB[32 * q:32 * q + 32, 128 * jb:128 * (jb + 1)]

            # One persistent staging buffer for the whole per-core output.
            # No slot recycling -> no release waits, so every copy carries
            # only its PE wait.
            stage = stagep.tile([128, TILES, T], fmm)

            # All 8 PSUM banks as one tile; matmul for tile k writes bank
            # k%8 (bank-aligned 512-col slots), copies read 4 banks at once.
            psum = psump.tile([128, 8, 512], f32)

            # local row = 128*p + tile: per-partition output is linear in
            # HBM, so writeback DMAs are long contiguous bursts per partition.
            out_lin = out_d.rearrange("(p r) t -> p r t", p=128, r=TILES)

            for j in range(TILES // 4):
                for q in range(4):
                    k = 4 * j + q
                    nc.tensor.matmul(
                        psum[:, k % 8, 0:T],
                        lhsT(j, q),
                        hrep[32 * q:32 * q + 32, :],
                        start=True,
                        stop=True,
                        tile_position=(32 * q, 0),
                    )
                    if q % 2 == 0:
                        continue
                    # 2-tile copy unit (prod tiles k-1, k): strided read
                    # across 2 PSUM banks into its permuted stage slot
                    u = k // 2
                    b = (2 * u) % 8
                    s = psum[:, b:b + 2, 0:T]
                    sl = PROD_STAGE[u]
                    dst = stage[:, 2 * sl:2 * sl + 2, :]
                    eng = PROD_ENG[u]
                    if eng == 0:
                        nc.vector.tensor_copy(dst, s)
                    elif eng == 1:
                        nc.scalar.copy(dst, s)
                    else:
                        nc.gpsimd.tensor_copy(dst, s)
                    for g0_, g1_ in DMA_FIRE.get(u, ()):
                        # All output groups on the single sync HWDGE ring:
                        # single-engine stage runs -> one wait per dma.
                        nc.sync.dma_start(
                            out_lin[:, 2 * g0_:2 * g1_, :],
                            stage[:, 2 * g0_:2 * g1_, :],
                        )

    # Walrus allows one sync wait per instruction. Single-engine stage
    # runs keep every output dma at one wait -- verify.
    sp_dmas = []
    for f in nc.m.functions:
        for b in f.blocks:
            for ins in b.instructions:
                s = str(ins)
                if ("DMACopy" in s and s.strip().startswith("SP")
                        and "@out" in s):
                    sp_dmas.append(ins)
    assert len(sp_dmas) == len(GROUPS), len(sp_dmas)
    for dm in sp_dmas:
        assert dm.sync_info is not None and len(list(dm.sync_info.on_wait)) == 1

    # Copies read PSUM banks guarded twice: by their own matmuls
    # (PE wait) and by the bank-WAR wait that the framework already placed
    # on the FIRST matmul of the unit (PE executes serially, so the copy's
    # PE wait transitively covers it). Strip the redundant non-PE wait.
    for f in nc.m.functions:
        for b in f.blocks:
            for ins in b.instructions:
                si = ins.sync_info
                if si is None:
                    continue
                waits = list(si.on_wait)
                if len(waits) <= 1:
                    continue
                pe = [w for w in waits if str(getattr(w, "ant_name", "")).startswith("PE")]
                assert len(waits) == 2 and len(pe) == 1, waits
                ins.sync_info = bass_rust.SyncInfo(
                    on_wait=pe, on_update=list(si.on_update)
                )

    _nc_cache.append(nc)
    return nc


def _run(in_maps, trace=False):
    from concourse.bass_utils import run_bass_kernel_spmd

    nc = _build_bass()
    return run_bass_kernel_spmd(nc, in_maps, list(range(NCORES)), trace=trace)


def kernel(x, c=None, sigma2=None, scale=None, _trace=False):
    vts, hc = _pack_inputs(x, c, sigma2, scale)
    acols = 128 * A_QUADS
    mcols = 128 * (A_QUADS + B_QUADS)
    in_maps = [
        {
            "va": np.ascontiguousarray(
                np.concatenate([vts[i][:, :acols], hc], axis=1)
            ),
            "vm": np.ascontiguousarray(vts[i][:, acols:mcols]),
            "vb": np.ascontiguousarray(vts[i][:, mcols:]),
        }
        for i in range(NCORES)
    ]
    res = _run(in_maps, trace=_trace)
    out = np.concatenate(
        [np.asarray(res.results[i]["out"]) for i in range(NCORES)], axis=0
    ).astype(np.float32)
    out = out.reshape(BATCH, DOF, T)
    if _trace:
        return out, res
    return out


# revision 38
# speedup vs baseline: 1.0419x; 1.0419x over previous
"""Trainium2 Bass kernel for the DMP-rollout Net (nn_Net_60567628808344).

Math
----
The reference integrates, per row r of p = (x*scale).reshape(-1, 27):
    y0 = p[:,0], goal = p[:,1], w = p[:,2:]
    cx_j = (1 - A_X*DT/TAU)^j                     (data independent)
    psi_j = exp(-0.5 (cx_j - c)^2 / sigma2)       (data independent)
    state update is LINEAR:  s_j = M s_{j-1} + [0; k*(Az*Bz*goal + F_j)]
    with F_j = (w @ psi_j) * cx_j * (goal-y0) / sum(psi_j)
So the whole 301-step rollout collapses to a closed form
    out[r, i] = A[i]*y0_r + B[i]*goal_r + (goal_r - y0_r) * (w_r @ H[i, :])
with constant A,B (301,), H (301, 25) precomputed in float64 on host.

Device work per core (8-way batch data-parallel) is then a single
(16384 x 27) @ (27 x 301) matmul + the 19.7 MB/core output writeback,
which is HBM-write roofline bound (~430 GB/s across the 16 DMA engines).

Inputs are bf16 (tolerance is 2e-2; bf16 keeps rel err ~5e-3) so the PE
produces well above the DMA drain rate, and the input load halves.

The device writes the output in bf16 (9.9 MB/core instead of 19.7; the
host casts back to fp32 after the gather -- adds ~0.4% rounding, total
rel err ~9e-3 vs the 2e-2 gate), which halves the HBM-write roofline.

PSUM is one persistent [128, 8, 512] tile (all 8 banks); matmul for tile
k writes bank k%8. PSUM->SBUF copies (with fp32->bf16 cast) move TWO
tiles per instruction via one strided read across 2 banks (4 units live
across 8 banks kills the copy->matmul->copy bank round-trip). A host-
side row permutation decouples production order from HBM order: each
output DMA group is a contiguous single-ENGINE run in stage space (one
sync wait per dma, the walrus ISA limit), while in time the DVE and ACT
chains interleave (GPSIMD cannot read PSUM). Production is the floor:
both copy engines run gap-free at instruction throughput.

Layout: per-core rows are packed 4 tiles at a time into the partition
dim (4 groups of 32 partitions, K padded 27->32 with zeros) so each
quad issues 4 row-group-packed concurrent matmuls (tile_position
(32q, 0) auto-derived from base partitions).
"""

import numpy as np
import ml_dtypes

# DMP hyperparameters fixed by Net.__init__ (hardcoded per problem spec)
N = 25
DOF = 2
DT = 0.01
TAU = 3.0
A_X = 2.0
A_Z = 48.0
B_Z = A_Z / 4.0
T = 301                    # time steps
BATCH = 65536
PARAM_DIM = DOF * (N + 2)  # 54
NCORES = 8

ROWS = BATCH * DOF         # 131072 (B*DOF rows)
RPC = ROWS // NCORES       # 16384 rows per core
TILES = RPC // 128         # 128 tiles of 128 rows per core
UNITS = TILES // 2         # 64 copy units of 2 tiles (half a j iteration)
QUAD_COLS = RPC // 4       # 4096: vt free dim (4 tiles packed in partition dim)

# Copy units are 2 tiles (2 PSUM banks): 4 units live across the 8 banks
# breaks the copy->matmul->copy bank round-trip that 4-tile units suffer
# (only 2 in flight), so each engine's copies run back-to-back.
#
# Production order is DECOUPLED from HBM order by a host-side row
# permutation: each output DMA group is a contiguous single-ENGINE run in
# stage space (single sync wait, no dma pairing), while in time the copy
# engines (DVE ~0.79us, ACT ~0.71us per 2-tile unit) work interleaved.
# GPSIMD cannot access PSUM (BIR verifier), so only DVE/ACT copy. 7 finer
# single-writer groups beat 4 paired ones: the last group is small, so
# the tail drain after production-end shrinks. Sizes from the gate model.
GROUPS = ((1, 7), (0, 10), (1, 9), (0, 10), (1, 9), (0, 9), (1, 10))
assert sum(n for _, n in GROUPS) == UNITS

# per-engine totals and stage-slot lists
_slots = {0: [], 1: [], 2: []}
_s0 = 0
for _e, _n in GROUPS:
    _slots[_e] += list(range(_s0, _s0 + _n))
    _s0 += _n
_TOT = {e: len(s) for e, s in _slots.items()}
# production order: weighted round-robin (Bresenham) over engines
_sched = sorted(
    ((k + 0.5) * UNITS / _TOT[e], e) for e in _slots for k in range(_TOT[e])
)
PROD_ENG = tuple(e for _, e in _sched)          # engine of production unit u
_ord = {0: 0, 1: 0, 2: 0}
PROD_STAGE = []                                  # stage slot of prod unit u
for _e in PROD_ENG:
    PROD_STAGE.append(_slots[_e][_ord[_e]])
    _ord[_e] += 1
PROD_STAGE = tuple(PROD_STAGE)
# dma trigger: group g fires after the production unit that completes it
_cum = {0: 0, 1: 0, 2: 0}
_fire = {}
_s0 = 0
for _gi, (_e, _n) in enumerate(GROUPS):
    _cum[_e] += _n
    _last_prod = [u for u in range(UNITS)
                  if PROD_ENG[u] == _e][_cum[_e] - 1]
    _fire.setdefault(_last_prod, []).append((_s0, _s0 + _n))
    _s0 += _n
DMA_FIRE = {u: tuple(v) for u, v in _fire.items()}
# tile-level permutation for host packing: production tile k -> stage tile
ST_TILE = tuple(2 * PROD_STAGE[k // 2] + k % 2 for k in range(TILES))

A_QUADS = 2                # quads in the tiny head chunk (with hc)
B_QUADS = 4                # quads in the second chunk (SWDGE)


# ----------------------------------------------------------------------------
# Host-side constant build (exact, float64)
# ----------------------------------------------------------------------------
_const_cache = {}


def _build_constants(c=None, sigma2=None):
    """Return hc (128, 301) bf16: rows 32q+p hold
    p==0 -> A, p==1 -> B, p==2+n -> H[:, n], rows 27..31 of each group zero."""
    if c is None:
        c = np.exp(-A_X * np.linspace(0.0, 1.0, N))
    if sigma2 is None:
        sigma2 = (N ** 1.5) / c / A_X
    c = np.asarray(c, np.float64)
    sigma2 = np.asarray(sigma2, np.float64)
    key = (c.tobytes(), sigma2.tobytes())
    if key in _const_cache:
        return _const_cache[key]

    k = DT / TAU
    M = np.array([[1.0, k], [-A_Z * B_Z * k, 1.0 - A_Z * k]])
    P = np.zeros(T + 1)
    Q = np.zeros(T + 1)
    Mn = np.eye(2)
    for n in range(T + 1):
        P[n] = Mn[0, 0]
        Q[n] = Mn[0, 1]
        Mn = Mn @ M

    decay = 1.0 - A_X * DT / TAU
    cx = decay ** np.arange(1, T + 1)                        # cx_1..cx_T
    psi = np.exp(-0.5 * (cx[:, None] - c[None, :]) ** 2 / sigma2[None, :])
    g = psi * (cx / psi.sum(1))[:, None]                     # (T, N)

    A = P[1:T + 1]
    B = k * A_Z * B_Z * np.cumsum(Q[0:T])
    # H[i] = k * sum_{m<=i} Q[i-m] g[m]  -- lower-triangular Toeplitz matvec
    ii = np.arange(T)[:, None]
    mm = np.arange(T)[None, :]
    L = np.where(ii >= mm, Q[np.clip(ii - mm, 0, T)], 0.0)   # (T, T)
    H = k * (L @ g)                                          # (T, N)

    hfull = np.zeros((32, T), np.float32)
    hfull[0] = A.astype(np.float32)
    hfull[1] = B.astype(np.float32)
    hfull[2:2 + N] = H.T.astype(np.float32)
    hc = np.tile(hfull, (4, 1)).astype(ml_dtypes.bfloat16)   # (128, T)
    _const_cache[key] = hc
    return hc


def _pack_inputs(x, c, sigma2, scale):
    """Build per-core vt arrays (128, 4096) bf16 + shared hc (128, 301)."""
    x = np.asarray(x, np.float32)
    if scale is None:
        scale = np.ones(PARAM_DIM, np.float32)
    p = (x * np.asarray(scale, np.float32)).reshape(ROWS, N + 2)
    y0 = p[:, 0]
    goal = p[:, 1]
    u = goal - y0
    v = np.empty((ROWS, N + 2), np.float32)
    v[:, 0] = y0
    v[:, 1] = goal
    v[:, 2:] = p[:, 2:] * u[:, None]
    v = v.astype(ml_dtypes.bfloat16)

    hc = _build_constants(c, sigma2)

    vts = []
    for i in range(NCORES):
        vc = v[RPC * i:RPC * (i + 1)]                 # (16384, 27)
        # Tile t=4j+q, lhsT free index f computes local row 128*f + t, so
        # each out-partition owns a contiguous run of HBM rows (linear
        # writeback).  row = 128*f + 4*j + q -> [f, j, q, p] -> [q, p, j, f]
        vperm = vc.reshape(128, TILES, N + 2)[:, ST_TILE, :]
        v4 = vperm.reshape(128, TILES // 4, 4, N + 2).transpose(2, 3, 1, 0)
        vp = np.zeros((4, 32, TILES // 4, 128), ml_dtypes.bfloat16)
        vp[:, :N + 2] = v4
        vts.append(np.ascontiguousarray(vp.reshape(128, QUAD_COLS)))
    return vts, hc


# ----------------------------------------------------------------------------
# Bass kernel
# ----------------------------------------------------------------------------
_nc_cache = []


def _build_bass():
    if _nc_cache:
        return _nc_cache[0]
    import concourse.bass as bass
    import concourse.mybir as mybir
    from concourse import tile
    import bass_rust
    from concourse.vector_clock import ScopedClock

    class SplitDrainTileContext(tile.TileContext):
        """This walrus build allows a single sync wait per instruction, but
        TileContext's kernel-tail drain carries one wait per live sem lane.
        Split the extras onto standalone single-wait SP nops (same stream, so
        all waits still complete before the barrier + sem clearing)."""

        def _drain_and_barrier(self, tick_clock, wait_clock):
            nc = self.nc
            drain_inst = nc.sync.drain()
            wait_clock.add_sem_waits(
                drain_inst.ins, ScopedClock({None: tick_clock.global_clock})
            )
            si = drain_inst.ins.sync_info
            waits = list(si.on_wait) if si is not None else []
            if len(waits) > 1:
                drain_inst.ins.sync_info = bass_rust.SyncInfo(
                    on_wait=[waits[0]], on_update=list(si.on_update)
                )
                engs = [nc.vector, nc.scalar, nc.tensor, nc.gpsimd, nc.sync]
                for wi, w in enumerate(waits[1:]):
                    n = engs[wi % len(engs)].nop(nofuse=True)
                    n.ins.sync_info = bass_rust.SyncInfo(
                        on_wait=[w], on_update=[]
                    )
            nc.all_engine_barrier()
            assert self.sems is not None
            popped = nc._tile_sem_poison_stack.pop()
            assert popped is self._sem_poison
            nc.clear_and_free_semaphores(list(self.sems.allocated().values()))
            nc.all_engine_barrier()

    f32 = mybir.dt.float32
    fmm = mybir.dt.bfloat16
    nc = bass.Bass()
    # Input split: a small head chunk (first A_QUADS quads + the 301 constant
    # columns) so compute starts early, the rest in parallel. Both ride the
    # gpsimd SWDGE so all 8 HWDGE sem lanes are free for output groups.
    va_d = nc.dram_tensor("va", [128, 128 * A_QUADS + T], fmm, kind="ExternalInput")
    vm_d = nc.dram_tensor("vm", [128, 128 * B_QUADS], fmm, kind="ExternalInput")
    vb_d = nc.dram_tensor(
        "vb", [128, QUAD_COLS - 128 * (A_QUADS + B_QUADS)], fmm,
        kind="ExternalInput"
    )
    out_d = nc.dram_tensor("out", [RPC, T], fmm, kind="ExternalOutput")

    with SplitDrainTileContext(nc) as tc:
        with (
            tc.tile_pool(name="vtp", bufs=1) as vtp,
            tc.tile_pool(name="stage", bufs=1) as stagep,
            tc.tile_pool(name="psum", bufs=1, space="PSUM") as psump,
        ):
            vtsA = vtp.tile([128, 128 * A_QUADS + T], fmm, tag="vtsA")
            vtsM = vtp.tile([128, 128 * B_QUADS], fmm, tag="vtsM")
            vtsB = vtp.tile(
                [128, QUAD_COLS - 128 * (A_QUADS + B_QUADS)], fmm, tag="vtsB"
            )
            nc.scalar.dma_start(vtsA[:], va_d[:])
            nc.gpsimd.dma_start(vtsM[:], vm_d[:])
            nc.gpsimd.dma_start(vtsB[:], vb_d[:])
            hrep = vtsA[:, 128 * A_QUADS:128 * A_QUADS + T]

            def lhsT(j, q):
                if j < A_QUADS:
                    return vtsA[32 * q:32 * q + 32, 128 * j:128 * (j + 1)]
                if j < A_QUADS + B_QUADS:
                    jm = j - A_QUADS
                    return vtsM[32 * q:32 * q + 32, 128 * jm:128 * (jm + 1)]
                jb = j - A_QUADS - B_QUADS
                return vtsB[32 * q:32 * q + 32, 128 * jb:128 * (jb + 1)]

            # One persistent staging buffer for the whole per-core output.
            # No slot recycling -> no release waits, so every copy carries
            # only its PE wait.
            stage = stagep.tile([128, TILES, T], fmm)

            # All 8 PSUM banks as one tile; matmul for tile k writes bank
            # k%8 (bank-aligned 512-col slots), copies read 4 banks at once.
            psum = psump.tile([128, 8, 512], f32)

            # local row = 128*p + tile: per-partition output is linear in
            # HBM, so writeback DMAs are long contiguous bursts per partition.
            out_lin = out_d.rearrange("(p r) t -> p r t", p=128, r=TILES)

            for j in range(TILES // 4):
                for q in range(4):
                    k = 4 * j + q
                    nc.tensor.matmul(
                        psum[:, k % 8, 0:T],
                        lhsT(j, q),
                        hrep[32 * q:32 * q + 32, :],
                        start=True,
                        stop=True,
                        tile_position=(32 * q, 0),
                    )
                    if q % 2 == 0:
                        continue
                    # 2-tile copy unit (prod tiles k-1, k): strided read
                    # across 2 PSUM banks into its permuted stage slot
                    u = k // 2
                    b = (2 * u) % 8
                    s = psum[:, b:b + 2, 0:T]
                    sl = PROD_STAGE[u]
                    dst = stage[:, 2 * sl:2 * sl + 2, :]
                    eng = PROD_ENG[u]
                    if eng == 0:
                        nc.vector.tensor_copy(dst, s)
                    elif eng == 1:
                        nc.scalar.copy(dst, s)
                    else:
                        nc.gpsimd.tensor_copy(dst, s)
                    for g0_, g1_ in DMA_FIRE.get(u, ()):
                        # All output groups on the single sync HWDGE ring:
                        # single-engine stage runs -> one wait per dma.
                        nc.sync.dma_start(
                            out_lin[:, 2 * g0_:2 * g1_, :],
                            stage[:, 2 * g0_:2 * g1_, :],
                        )

    # Walrus allows one sync wait per instruction. Single-engine stage
    # runs keep every output dma at one wait -- verify.
    sp_dmas = []
    for f in nc.m.functions:
        for b in f.blocks:
            for ins in b.instructions:
                s = str(ins)
                if ("DMACopy" in s and s.strip().startswith("SP")
                        and "@out" in s):
                    sp_dmas.append(ins)
    assert len(sp_dmas) == len(GROUPS), len(sp_dmas)
    for dm in sp_dmas:
        assert dm.sync_info is not None and len(list(dm.sync_info.on_wait)) == 1

    # Copies read PSUM banks guarded twice: by their own matmuls
    # (PE wait) and by the bank-WAR wait that the framework already placed
    # on the FIRST matmul of the unit (PE executes serially, so the copy's
    # PE wait transitively covers it). Strip the redundant non-PE wait.
    for f in nc.m.functions:
        for b in f.blocks:
            for ins in b.instructions:
                si = ins.sync_info
                if si is None:
                    continue
                waits = list(si.on_wait)
                if len(waits) <= 1:
                    continue
                pe = [w for w in waits if str(getattr(w, "ant_name", "")).startswith("PE")]
                assert len(waits) == 2 and len(pe) == 1, waits
                ins.sync_info = bass_rust.SyncInfo(
                    on_wait=pe, on_update=list(si.on_update)
                )

    _nc_cache.append(nc)
    return nc


def _run(in_maps, trace=False):
    from concourse.bass_utils import run_bass_kernel_spmd

    nc = _build_bass()
    return run_bass_kernel_spmd(nc, in_maps, list(range(NCORES)), trace=trace)


def kernel(x, c=None, sigma2=None, scale=None, _trace=False):
    vts, hc = _pack_inputs(x, c, sigma2, scale)
    acols = 128 * A_QUADS
    mcols = 128 * (A_QUADS + B_QUADS)
    in_maps = [
        {
            "va": np.ascontiguousarray(
                np.concatenate([vts[i][:, :acols], hc], axis=1)
            ),
            "vm": np.ascontiguousarray(vts[i][:, acols:mcols]),
            "vb": np.ascontiguousarray(vts[i][:, mcols:]),
        }
        for i in range(NCORES)
    ]
    res = _run(in_maps, trace=_trace)
    out = np.concatenate(
        [np.asarray(res.results[i]["out"]) for i in range(NCORES)], axis=0
    ).astype(np.float32)
    out = out.reshape(BATCH, DOF, T)
    if _trace:
        return out, res
    return out
